# revision 9
# baseline (speedup 1.0000x reference)
"""Multi-head self-attention kernel for Trainium2 (8 NeuronCores).

Problem: q,k,v [4000, 4096] fp32; the module attends q against itself
(k and v are ignored by the reference). 32 heads of dim 128.

Sharding: tensor-parallel over heads - each of the 8 cores owns 4 heads
(a [4000, 512] column slice of q) and computes its full attention
independently; the host concatenates the per-core outputs (the
"all-gather" along the feature axis).

v2 algorithm (per head, all fp16 compute):
  The score matrix S = q q^T / sqrt(hd) is SYMMETRIC, so P = exp(S*scale - 8.5)
  is symmetric too. Only the upper-triangle 128x128 blocks are computed:
    - mm1 row-strips: S_T[c, c..31] on the PE (fp16, 1 cycle/row at any width)
    - exp on ACT into fp16 strips (halves the ACT work vs the full matrix)
    - the lower-triangle tiles are produced by DMA-engine blockwise
      transposes of the fp16 strips (InstDmaTransposeAnt, idle device)
  mm2 runs in natural orientation: O[q in unit u, :] = sum_r tile(r,u)^T @ vr[r]
  with vr[r] = [v block r | ones column]. The ones column makes the PSUM
  accumulator's last column the softmax denominator l[q] exactly (the -8.5
  shift cancels in the ratio), so there is no separate row-sum pass and no
  output transpose epilogue: out = po[:, :128] * (1/po[:, 128]).
"""

import numpy as np

N = 4000
D_MODEL = 4096
NUM_HEADS = 32
HD = 128
N_CORES = 8
H_PER_CORE = NUM_HEADS // N_CORES          # 4
D_CORE = H_PER_CORE * HD                   # 512
P = 128
U = 32                                     # 32 row/col units of 128 (4096 padded)
NPAD = U * P                               # 4096
SCALE = 1.0 / np.sqrt(np.float32(HD))
EXP_SHIFT = 8.5

_CACHE = {}


def _build():
    import concourse.bacc as bacc
    import concourse.tile as tile
    from concourse import mybir

    f32 = mybir.dt.float32
    fp16 = mybir.dt.float16
    Exp = mybir.ActivationFunctionType.Exp

    nc = bacc.Bacc("TRN2", target_bir_lowering=False, debug=False)
    q_in = nc.declare_dram_parameter("q", [N, D_CORE], f32, isOutput=False)
    o_out = nc.declare_dram_parameter("out", [N, D_CORE], f32, isOutput=True)

    with tile.TileContext(nc) as tc:
        with (
            tc.tile_pool(name="singles", bufs=1) as singles,
            tc.tile_pool(name="qn16", bufs=1) as qn_pool,
            tc.tile_pool(name="qT", bufs=1) as qT_pool,
            tc.tile_pool(name="vr", bufs=1) as vr_pool,
            tc.tile_pool(name="strip", bufs=1) as strip_pool,
            tc.tile_pool(name="mirE", bufs=2) as mirE_pool,
            tc.tile_pool(name="mirO", bufs=1) as mirO_pool,
            tc.tile_pool(name="ob", bufs=2) as ob_pool,
            tc.tile_pool(name="rrec", bufs=4) as r_pool,
            tc.tile_pool(name="ps_s", bufs=2, space="PSUM") as ps_s_pool,
            tc.tile_pool(name="ps_o", bufs=2, space="PSUM") as ps_o_pool,
        ):
            exp_bias = singles.tile([P, 1], f32)
            nc.vector.memset(exp_bias, -float(EXP_SHIFT))

            # vr_all[:, r, 0:128] = q rows of block r (fp16), col 128 = 1.0
            # (0 in the padding rows of the last block so padded keys add
            # nothing to the softmax denominator).
            vr_all = vr_pool.tile([P, U, HD + 1], fp16, tag="vr")
            nc.vector.memset(vr_all[:, :, HD : HD + 1], 1.0)
            nc.vector.memset(vr_all[N - (U - 1) * P :, U - 1, HD : HD + 1], 0.0)

            def emit_load(h):
                """DMA q[:, head cols] f32 -> qn16 [128, 4096] fp16 (cast via
                gpsimd SWDGE). 4 chunks of 8 blocks + zero-padded tail."""
                hs = slice(h * HD, (h + 1) * HD)
                qn = qn_pool.tile([P, NPAD], fp16, tag="qn")
                for ch in range(4):
                    b0 = ch * 8
                    b1 = U - 1 if ch == 3 else b0 + 8  # last block separate
                    nc.gpsimd.dma_start(
                        out=qn[:, b0 * P : b1 * P].rearrange(
                            "p (c d) -> p c d", c=b1 - b0
                        ),
                        in_=q_in[b0 * P : b1 * P, hs].rearrange(
                            "(c p) d -> p c d", p=P
                        ),
                    )
                nc.vector.memset(qn[:, (U - 1) * P :], 0.0)
                nc.gpsimd.dma_start(
                    out=qn[: N - (U - 1) * P, (U - 1) * P :],
                    in_=q_in[(U - 1) * P : N, hs],
                )
                return qn

            def emit_vr(qn):
                """vr_all value columns from qn (fp16->fp16 DVE copies)."""
                for r in range(U):
                    nc.vector.tensor_copy(
                        vr_all[:, r, 0:HD], qn[:, r * P : (r + 1) * P]
                    )

            def emit_qT(qn):
                """qT [hd, 4096] fp16 via blockwise DMA transpose of qn."""
                qT = qT_pool.tile([P, NPAD], fp16, tag="qT")
                for k in range(4):
                    cs = slice(k * 8 * P, (k + 1) * 8 * P)
                    nc.sync.dma_start(
                        out=qT[:, cs].rearrange("p (c d) -> p c d", c=8),
                        in_=qn[:, cs],
                        transpose=True,
                    )
                return qT

            strips = {}
            mirs = {}

            def emit_mirror(u, pool, tag):
                """mirror transpose of strip u minus its diagonal block:
                the lower-triangle tiles [k-part, u-cols] for k > u."""
                mir = pool.tile([P, (U - 1 - (u % 4)) * P], fp16, tag=tag)
                nc.sync.dma_start(
                    out=mir[:, : (U - 1 - u) * P].rearrange(
                        "p (c d) -> p c d", c=U - 1 - u
                    ),
                    in_=strips[u][:, P:],
                    transpose=True,
                )
                mirs[u] = mir

            def emit_stage(qT, g):
                """mm1 + exp for rows 4g..4g+3 (upper triangle strips);
                mirrors for the first two rows issued here (one band early)
                so their latency hides under the previous band's mm2."""
                for c in range(4 * g, 4 * g + 4):
                    W = (U - c) * P
                    st = strip_pool.tile([P, W], fp16, tag=f"strip{c}")
                    strips[c] = st
                    off = 0
                    while off < W:
                        w = min(1024, W - off)
                        ps = ps_s_pool.tile([P, 1024], f32, tag="ps_s")
                        o2 = 0
                        while o2 < w:
                            ww = min(512, w - o2)
                            nc.tensor.matmul(
                                ps[:, o2 : o2 + ww],
                                lhsT=qT[:, c * P : (c + 1) * P],
                                rhs=qT[:, c * P + off + o2 : c * P + off + o2 + ww],
                                start=True,
                                stop=True,
                            )
                            o2 += ww
                        nc.scalar.activation(
                            st[:, off : off + w],
                            ps[:, :w],
                            Exp,
                            scale=float(SCALE),
                            bias=exp_bias[:, :],
                        )
                        off += w
                    if c % 4 < 2 and c < U - 1:
                        emit_mirror(c, mirE_pool, f"me{c % 4}")

            def emit_band_mm2(h, g):
                """per unit u in band g: 32-step mm2 accumulation + epilogue;
                then the band's output DMA (on gpsimd so its sem wait doesn't
                block the ACT sequencer)."""
                hs = slice(h * HD, (h + 1) * HD)
                for j in (2, 3):
                    u = 4 * g + j
                    if u < U - 1:
                        emit_mirror(u, mirO_pool, f"mo{j}")
                ob = ob_pool.tile([P, 4 * P], f32, tag="ob")
                for j in range(4):
                    u = 4 * g + j
                    mir = mirs.get(u)
                    po = ps_o_pool.tile([P, 512], f32, tag="ps_o")
                    for r in range(U):
                        if r <= u:
                            lhsT = strips[r][:, (u - r) * P : (u - r + 1) * P]
                        else:
                            lhsT = mir[:, (r - u - 1) * P : (r - u) * P]
                        nc.tensor.matmul(
                            po[:, 0 : HD + 1],
                            lhsT=lhsT,
                            rhs=vr_all[:, r, :],
                            start=(r == 0),
                            stop=(r == U - 1),
                        )
                    r_t = r_pool.tile([P, 1], f32, tag=f"r{u % 4}")
                    nc.vector.reciprocal(r_t, po[:, HD : HD + 1])
                    nc.vector.tensor_scalar_mul(
                        ob[:, j * P : (j + 1) * P], po[:, 0:HD], r_t[:, 0:1]
                    )
                # write band g output rows
                if g < 7:
                    nc.gpsimd.dma_start(
                        out=o_out[g * 512 : (g + 1) * 512, hs].rearrange(
                            "(c p) d -> p c d", p=P
                        ),
                        in_=ob.rearrange("p (c d) -> p c d", c=4),
                    )
                else:
                    nc.gpsimd.dma_start(
                        out=o_out[7 * 512 : 7 * 512 + 3 * P, hs].rearrange(
                            "(c p) d -> p c d", p=P
                        ),
                        in_=ob[:, 0 : 3 * P].rearrange("p (c d) -> p c d", c=3),
                    )
                    nc.gpsimd.dma_start(
                        out=o_out[31 * P : N, hs],
                        in_=ob[: N - 31 * P, 3 * P : 4 * P],
                    )

            qn = emit_load(0)
            emit_vr(qn)
            qT = emit_qT(qn)
            for h in range(H_PER_CORE):
                emit_stage(qT, 0)
                emit_stage(qT, 1)
                for g in range(8):
                    emit_band_mm2(h, g)
                    if g + 2 <= 7:
                        emit_stage(qT, g + 2)
                    if g == 0 and h + 1 < H_PER_CORE:
                        qn = emit_load(h + 1)
                if h + 1 < H_PER_CORE:
                    emit_vr(qn)
                    qT = emit_qT(qn)

    nc.compile()
    return nc


def _get_nc():
    if "nc" not in _CACHE:
        _CACHE["nc"] = _build()
    return _CACHE["nc"]


def _get_runner():
    """Build (once) a jitted 8-core SPMD executor for the compiled program.

    Mirrors concourse.bass2jax.run_bass_via_pjrt but caches the jitted
    callable so repeat kernel() calls skip retracing/recompilation.
    """
    if "runner" in _CACHE:
        return _CACHE["runner"]

    import jax
    import numpy as _np
    from jax.sharding import Mesh, PartitionSpec
    from jax.experimental.shard_map import shard_map
    from concourse import mybir
    from concourse import bass2jax

    nc = _get_nc()
    bass2jax.install_neuronx_cc_hook()

    in_names, out_names, out_avals, zero_outs = [], [], [], []
    for alloc in nc.m.functions[0].allocations:
        if not isinstance(alloc, mybir.MemoryLocationSet):
            continue
        name = alloc.memorylocations[0].name
        pname = nc.partition_id_tensor.name if nc.partition_id_tensor else None
        if alloc.kind == "ExternalInput":
            if name != pname:
                in_names.append(name)
        elif alloc.kind == "ExternalOutput":
            shape = tuple(alloc.tensor_shape)
            dtype = mybir.dt.np(alloc.dtype)
            out_names.append(name)
            out_avals.append(jax.core.ShapedArray(shape, dtype))
            zero_outs.append((shape, dtype))
    n_params = len(in_names)
    n_outs = len(out_avals)
    all_names = in_names + out_names
    pname = nc.partition_id_tensor.name if nc.partition_id_tensor else None
    if pname is not None:
        all_names = all_names + [pname]

    def _body(*args):
        operands = list(args)
        if pname is not None:
            operands.append(bass2jax.partition_id_tensor())
        outs = bass2jax._bass_exec_p.bind(
            *operands,
            out_avals=tuple(out_avals),
            in_names=tuple(all_names),
            out_names=tuple(out_names),
            lowering_input_output_aliases=(),
            sim_require_finite=True,
            sim_require_nnan=True,
            nc=nc,
        )
        return tuple(outs)

    devices = jax.devices()[:N_CORES]
    mesh = Mesh(_np.asarray(devices), ("core",))
    specs = (PartitionSpec("core"),) * (n_params + n_outs)
    sharded = jax.jit(
        shard_map(
            _body,
            mesh=mesh,
            in_specs=specs,
            out_specs=(PartitionSpec("core"),) * n_outs,
            check_rep=False,
        ),
        donate_argnums=tuple(range(n_params, n_params + n_outs)),
        keep_unused=True,
    )

    def run(per_core_inputs):
        concat_in = [
            _np.concatenate([m[nm] for m in per_core_inputs], axis=0)
            for nm in in_names
        ]
        concat_zero = [
            _np.zeros((N_CORES * s[0], *s[1:]), dt) for s, dt in zero_outs
        ]
        out_arrs = sharded(*concat_in, *concat_zero)
        return [
            {
                nm: _np.asarray(out_arrs[i]).reshape(
                    N_CORES, *out_avals[i].shape
                )[c]
                for i, nm in enumerate(out_names)
            }
            for c in range(N_CORES)
        ]

    _CACHE["runner"] = run
    return run


def kernel(**inputs: np.ndarray) -> np.ndarray:
    q = np.ascontiguousarray(np.asarray(inputs["q"], dtype=np.float32))
    assert q.shape == (N, D_MODEL)

    in_maps = [
        {"q": np.ascontiguousarray(q[:, c * D_CORE : (c + 1) * D_CORE])}
        for c in range(N_CORES)
    ]
    try:
        run = _get_runner()
        results = run(in_maps)
    except Exception:
        # fall back to the stock SPMD runner (pays a re-jit per call)
        from concourse.bass_utils import run_bass_kernel_spmd

        _CACHE.pop("runner", None)
        res = run_bass_kernel_spmd(_get_nc(), in_maps, list(range(N_CORES)))
        results = res.results
    out = np.concatenate([results[c]["out"] for c in range(N_CORES)], axis=1)
    return out.astype(np.float32)
